# revision 2
# baseline (speedup 1.0000x reference)
"""Binary 3x3 conv (sign(x) * sign(w) conv, scaled by alpha) on 8 TRN2 NeuronCores.

Strategy
--------
- Data-parallel over batch: 32 images -> 4 per core; weights replicated.
- Conv lowered to 9 shifted matmuls accumulating in PSUM, contracting over
  input channels (C=256) placed on SBUF partitions (2 chunks of 128).
- sign(x), sign(w) computed on-chip (ScalarE Sign activation) directly into
  fp8e4m3 — ±1/0 are exact in fp8, products are ±1/0, PSUM accumulates in
  fp32, sums ≤ 2304 are exact integers -> bit-exact result.
- fp8 DoubleRow perf mode packs both 128-channel chunks into one matmul
  (effective K=256, 2 MACs/cell/cycle).
- Activation planes stored W-padded (58 wide) and H-padded (rows 0/57 zero),
  so every 3x3 tap window is a *contiguous* 1-D span of the flattened plane.
  We compute 58 output columns per row (2 garbage) and drop the garbage
  columns during PSUM->SBUF eviction.
"""

import numpy as np

import concourse.bacc as bacc
import concourse.bass as bass
import concourse.mybir as mybir
from concourse import tile
from concourse.bass_utils import run_bass_kernel_spmd

N_CORES = 8
B, C, H, W = 32, 256, 56, 56
BP = B // N_CORES  # images per core
O = 256
PW = W + 2  # padded row width (58)
ROWS = 60  # rows per plane block: 58 used + 2 slack so cc-stride % 16 == 0
PLANE = ROWS * PW  # 3480 elems per (img) plane
CC_STRIDE = BP * PLANE  # 13920, stride between the two 128-channel chunks
GUARD = 16  # header so the (dy=-1,dx=-1) tap of plane 0 stays in-bounds
PAD_FREE = GUARD + 2 * CC_STRIDE  # fp8 elems per partition in the pad buffer

ROWS_PER_TILE = 8
NT = H // ROWS_PER_TILE  # 7 pixel tiles per image
FD = ROWS_PER_TILE * PW  # 464 matmul free dim (<=512: one PSUM bank)

F8 = mybir.dt.float8e4
F32 = mybir.dt.float32

_compiled = None


def _plane_base(cc: int, img: int) -> int:
    return GUARD + cc * CC_STRIDE + img * PLANE


def _build():
    nc = bacc.Bacc("TRN2", target_bir_lowering=False, debug=False, num_devices=N_CORES)

    x_dram = nc.dram_tensor("x", [BP, C, H, W], F32, kind="ExternalInput")
    wt_dram = nc.dram_tensor("wt", [C, 9, O], F32, kind="ExternalInput")
    alpha_dram = nc.dram_tensor("alpha", [1], F32, kind="ExternalInput")
    out_dram = nc.dram_tensor("out", [BP, O, H, W], F32, kind="ExternalOutput")

    with tile.TileContext(nc) as tc:
        with (
            tc.tile_pool(name="const", bufs=1) as const_pool,
            tc.tile_pool(name="xin", bufs=4) as xin_pool,
            tc.tile_pool(name="oplane", bufs=3) as out_pool,
            tc.tile_pool(name="psum", bufs=8, space=bass.MemorySpace.PSUM) as psum_pool,
        ):
            # alpha broadcast to all 128 partitions
            alpha_sb = const_pool.tile([128, 1], F32, name="alpha_sb")
            nc.sync.dma_start(alpha_sb[:], alpha_dram.ap().partition_broadcast(128))

            # weights: f32 [cchunk->partitions, cc, 9*O] -> sign -> fp8
            wsb = const_pool.tile([128, 2, 9 * O], F32, name="wsb")
            w8 = const_pool.tile([128, 2, 9 * O], F8, name="w8")
            for cc in range(2):
                nc.sync.dma_start(
                    wsb[:, cc, :],
                    wt_dram[cc * 128 : (cc + 1) * 128].rearrange("c s o -> c (s o)"),
                )
                nc.scalar.sign(w8[:, cc, :], wsb[:, cc, :])

            # padded fp8 activation planes
            pad = const_pool.tile([128, PAD_FREE], F8, name="pad")
            pad_h = pad[:].tensor
            pstep = pad[:].ap[0][0]

            # zero the halo regions of each plane (top row+guard, L/R pads,
            # bottom row + the sliver of slack row 58 the widest tap reads)
            for cc in range(2):
                for img in range(BP):
                    base = _plane_base(cc, img)
                    nc.gpsimd.memset(
                        bass.AP(pad_h, base - 1, [[pstep, 128], [1, PW + 1]]), 0
                    )
                    nc.gpsimd.memset(
                        bass.AP(pad_h, base + PW - 1, [[pstep, 128], [PW, PW - 1], [1, 2]]),
                        0,
                    )
                    nc.gpsimd.memset(
                        bass.AP(pad_h, base + 57 * PW, [[pstep, 128], [1, PW + 1]]), 0
                    )

            # load + binarize activations into the padded planes
            for img in range(BP):
                for cc in range(2):
                    xin = xin_pool.tile([128, H, W], F32, name="xin")
                    nc.sync.dma_start(xin[:], x_dram[img, cc * 128 : (cc + 1) * 128])
                    dst = bass.AP(
                        pad_h,
                        _plane_base(cc, img) + PW + 1,
                        [[pstep, 128], [PW, H], [1, W]],
                    )
                    nc.scalar.sign(dst, xin[:])

            # conv: 9 shifted fp8 DoubleRow matmuls per output tile
            w8_h = w8[:].tensor
            w8_step = w8[:].ap[0][0]
            for oc in range(2):
                for img in range(BP):
                    psums = [
                        psum_pool.tile([128, FD], F32, name="ps", tag="ps")
                        for _ in range(NT)
                    ]
                    for s in range(9):
                        dy, dx = s // 3 - 1, s % 3 - 1
                        lhsT = bass.AP(
                            w8_h,
                            s * O + oc * 128,
                            [[w8_step, 128], [9 * O, 2], [1, 128]],
                        )
                        for t in range(NT):
                            rhs = bass.AP(
                                pad_h,
                                GUARD
                                + img * PLANE
                                + (ROWS_PER_TILE * t + 1 + dy) * PW
                                + dx,
                                [[pstep, 128], [CC_STRIDE, 2], [1, FD]],
                            )
                            nc.tensor.matmul(
                                psums[t][:],
                                lhsT,
                                rhs,
                                start=(s == 0),
                                stop=(s == 8),
                                perf_mode=mybir.MatmulPerfMode.DoubleRow,
                            )
                    # evict: drop garbage columns, scale by alpha
                    oplane = out_pool.tile([128, H, W], F32, name="oplane")
                    for t in range(NT):
                        pbase = psums[t][:]
                        src = bass.AP(
                            pbase.tensor,
                            pbase.offset + 1,
                            [[pbase.ap[0][0], 128], [PW, ROWS_PER_TILE], [1, W]],
                        )
                        nc.vector.tensor_scalar_mul(
                            oplane[:, ROWS_PER_TILE * t : ROWS_PER_TILE * (t + 1), :],
                            src,
                            alpha_sb[:, 0:1],
                        )
                    nc.sync.dma_start(
                        out_dram[img, oc * 128 : (oc + 1) * 128], oplane[:]
                    )

    nc.compile()
    return nc


def _get_compiled():
    global _compiled
    if _compiled is None:
        _compiled = _build()
    return _compiled


def run(x: np.ndarray, weight: np.ndarray, alpha: np.ndarray, **kw):
    nc = _get_compiled()
    # [o,c,ky,kx] -> [c, ky*3+kx, o] so channels land on partitions directly
    wt = np.ascontiguousarray(weight.transpose(1, 2, 3, 0).reshape(C, 9, O)).astype(
        np.float32
    )
    x = np.ascontiguousarray(x, dtype=np.float32)
    alpha = np.ascontiguousarray(alpha, dtype=np.float32)
    in_maps = [
        {"x": x[i * BP : (i + 1) * BP], "wt": wt, "alpha": alpha}
        for i in range(N_CORES)
    ]
    res = run_bass_kernel_spmd(nc, in_maps, list(range(N_CORES)), **kw)
    return np.concatenate([r["out"] for r in res.results], axis=0), res


def kernel(x: np.ndarray, weight: np.ndarray, alpha: np.ndarray) -> np.ndarray:
    return run(x, weight, alpha)[0]


# revision 4
# speedup vs baseline: 1.1477x; 1.1477x over previous
"""Binary 3x3 conv (sign(x) * sign(w) conv, scaled by alpha) on 8 TRN2 NeuronCores.

Strategy
--------
- Data-parallel over batch: 32 images -> 4 per core; weights replicated.
- Conv lowered to 9 shifted matmuls accumulating in PSUM, contracting over
  input channels (C=256) placed on SBUF partitions (2 chunks of 128).
- sign(x), sign(w) computed on-chip (ScalarE Sign activation) directly into
  fp8e4m3 — ±1/0 are exact in fp8, products are ±1/0, PSUM accumulates in
  fp32, sums ≤ 2304 are exact integers -> bit-exact result.
- fp8 DoubleRow perf mode packs both 128-channel chunks into one matmul
  (effective K=256, 2 MACs/cell/cycle) -> 9*2*4*7 = 504 matmuls/core at
  ~197ns issue rate = ~100us PE floor.
- Activation planes stored W-padded (58 wide) and H-padded (rows 0/57 zero),
  so every 3x3 tap window is a *contiguous* 1-D span of the flattened plane.
  We compute 58 output columns per row (2 garbage) and drop the garbage
  columns during PSUM->SBUF eviction.
- One pad tile PER IMAGE so image-0 matmuls start as soon as image 0 is
  loaded+signed (Tile deps are tile-granular); dummy matmuls on a zero
  scratch tile warm the PE HAM clock gate during the prologue.
"""

import numpy as np

import concourse.bacc as bacc
import concourse.bass as bass
import concourse.mybir as mybir
from concourse import tile
from concourse.bass_utils import run_bass_kernel_spmd

N_CORES = 8
B, C, H, W = 32, 256, 56, 56
BP = B // N_CORES  # images per core
O = 256
PW = W + 2  # padded row width (58)
PLANE = 3488  # fp8 elems per (img, cc) plane; 58*58=3364 used, %16==0
GUARD = 16  # header so the (dy=-1,dx=-1) tap of cc0 stays in-bounds
PAD_FREE = GUARD + 2 * PLANE  # per-partition fp8 elems in one image's pad tile

ROWS_PER_TILE = 8
NT = H // ROWS_PER_TILE  # 7 pixel tiles per image
FD = ROWS_PER_TILE * PW  # 464 matmul free dim (<=512: one PSUM bank)

N_WARMUP_MM = 30  # dummy matmuls to lift the PE HAM clock gate to 2.4GHz

F8 = mybir.dt.float8e4
F32 = mybir.dt.float32

_compiled = None


def _build():
    nc = bacc.Bacc("TRN2", target_bir_lowering=False, debug=False, num_devices=N_CORES)

    x_dram = nc.dram_tensor("x", [BP, C, H, W], F32, kind="ExternalInput")
    wt_dram = nc.dram_tensor("wt", [C, 9, O], F32, kind="ExternalInput")
    alpha_dram = nc.dram_tensor("alpha", [1], F32, kind="ExternalInput")
    out_dram = nc.dram_tensor("out", [BP, O, H, W], F32, kind="ExternalOutput")

    with tile.TileContext(nc) as tc:
        with (
            tc.tile_pool(name="const", bufs=1) as const_pool,
            tc.tile_pool(name="xin", bufs=4) as xin_pool,
            tc.tile_pool(name="oplane", bufs=3) as out_pool,
            tc.tile_pool(name="psum", bufs=7, space=bass.MemorySpace.PSUM) as psum_pool,
            tc.tile_pool(name="wpsum", bufs=1, space=bass.MemorySpace.PSUM) as wpsum_pool,
        ):
            # --- PE warm-up: matmuls on a zeroed scratch tile, no data deps
            warm = const_pool.tile([128, 2, FD], F8, name="warm")
            nc.gpsimd.memset(warm[:], 0)
            wps = wpsum_pool.tile([128, FD], F32, name="wps")
            for _ in range(N_WARMUP_MM):
                nc.tensor.matmul(
                    wps[:],
                    warm[:, :, 0:128],
                    warm[:],
                    start=True,
                    stop=True,
                    perf_mode=mybir.MatmulPerfMode.DoubleRow,
                )

            # alpha broadcast to all 128 partitions
            alpha_sb = const_pool.tile([128, 1], F32, name="alpha_sb")
            nc.sync.dma_start(alpha_sb[:], alpha_dram.ap().partition_broadcast(128))

            # weights: f32 [cchunk->partitions, cc, 9*O] -> sign -> fp8.
            # DMA on the scalar-engine HWDGE ring so it runs parallel with the
            # x loads on the sync ring.
            wsb = const_pool.tile([128, 2, 9 * O], F32, name="wsb")
            w8 = const_pool.tile([128, 2, 9 * O], F8, name="w8")
            for cc in range(2):
                nc.scalar.dma_start(
                    wsb[:, cc, :],
                    wt_dram[cc * 128 : (cc + 1) * 128].rearrange("c s o -> c (s o)"),
                )
                nc.scalar.sign(w8[:, cc, :], wsb[:, cc, :])

            # per-image padded fp8 activation planes (both cc chunks in one
            # tile: the DoubleRow rhs AP needs a fixed stride between chunks)
            pads = []
            for img in range(BP):
                p = const_pool.tile([128, PAD_FREE], F8, name=f"pad{img}")
                pads.append(p)
                ph, pstep = p[:].tensor, p[:].ap[0][0]
                for cc in range(2):
                    base = GUARD + cc * PLANE
                    # top pad row (+ leading guard elem), L/R column pads,
                    # bottom pad row (+ the sliver the widest tap reads)
                    nc.gpsimd.memset(
                        bass.AP(ph, base - 1, [[pstep, 128], [1, PW + 1]]), 0
                    )
                    nc.gpsimd.memset(
                        bass.AP(ph, base + PW - 1, [[pstep, 128], [PW, PW - 1], [1, 2]]),
                        0,
                    )
                    nc.gpsimd.memset(
                        bass.AP(ph, base + 57 * PW, [[pstep, 128], [1, PW + 1]]), 0
                    )

            # load + binarize activations into the padded planes
            for img in range(BP):
                ph, pstep = pads[img][:].tensor, pads[img][:].ap[0][0]
                for cc in range(2):
                    xin = xin_pool.tile([128, H, W], F32, name="xin")
                    nc.sync.dma_start(xin[:], x_dram[img, cc * 128 : (cc + 1) * 128])
                    dst = bass.AP(
                        ph,
                        GUARD + cc * PLANE + PW + 1,
                        [[pstep, 128], [PW, H], [1, W]],
                    )
                    nc.scalar.sign(dst, xin[:])

            # conv: 9 shifted fp8 DoubleRow matmuls per output tile
            w8_h = w8[:].tensor
            w8_step = w8[:].ap[0][0]
            for img in range(BP):
                ph, pstep = pads[img][:].tensor, pads[img][:].ap[0][0]
                for oc in range(2):
                    psums = [
                        psum_pool.tile([128, FD], F32, name="ps", tag="ps")
                        for _ in range(NT)
                    ]
                    for s in range(9):
                        dy, dx = s // 3 - 1, s % 3 - 1
                        lhsT = bass.AP(
                            w8_h,
                            s * O + oc * 128,
                            [[w8_step, 128], [9 * O, 2], [1, 128]],
                        )
                        for t in range(NT):
                            rhs = bass.AP(
                                ph,
                                GUARD + (ROWS_PER_TILE * t + 1 + dy) * PW + dx,
                                [[pstep, 128], [PLANE, 2], [1, FD]],
                            )
                            nc.tensor.matmul(
                                psums[t][:],
                                lhsT,
                                rhs,
                                start=(s == 0),
                                stop=(s == 8),
                                perf_mode=mybir.MatmulPerfMode.DoubleRow,
                            )
                    # evict: drop garbage columns, scale by alpha; alternate
                    # DVE/ACT so a group drains in half the time
                    oplane = out_pool.tile([128, H, W], F32, name="oplane")
                    for t in range(NT):
                        pbase = psums[t][:]
                        src = bass.AP(
                            pbase.tensor,
                            pbase.offset + 1,
                            [[pbase.ap[0][0], 128], [PW, ROWS_PER_TILE], [1, W]],
                        )
                        dst = oplane[:, ROWS_PER_TILE * t : ROWS_PER_TILE * (t + 1), :]
                        if t % 2 == 0:
                            nc.vector.tensor_scalar_mul(dst, src, alpha_sb[:, 0:1])
                        else:
                            nc.scalar.mul(dst, src, alpha_sb[:, 0:1])
                    # split the store so it starts before the last eviction
                    half = (NT // 2) * ROWS_PER_TILE  # rows 0..23 / 24..55
                    och = out_dram[img, oc * 128 : (oc + 1) * 128]
                    nc.sync.dma_start(och[:, :half, :], oplane[:, :half, :])
                    nc.sync.dma_start(och[:, half:, :], oplane[:, half:, :])

    nc.compile()
    return nc


def _get_compiled():
    global _compiled
    if _compiled is None:
        _compiled = _build()
    return _compiled


def run(x: np.ndarray, weight: np.ndarray, alpha: np.ndarray, **kw):
    nc = _get_compiled()
    # [o,c,ky,kx] -> [c, ky*3+kx, o] so channels land on partitions directly
    wt = np.ascontiguousarray(weight.transpose(1, 2, 3, 0).reshape(C, 9, O)).astype(
        np.float32
    )
    x = np.ascontiguousarray(x, dtype=np.float32)
    alpha = np.ascontiguousarray(alpha, dtype=np.float32)
    in_maps = [
        {"x": x[i * BP : (i + 1) * BP], "wt": wt, "alpha": alpha}
        for i in range(N_CORES)
    ]
    res = run_bass_kernel_spmd(nc, in_maps, list(range(N_CORES)), **kw)
    return np.concatenate([r["out"] for r in res.results], axis=0), res


def kernel(x: np.ndarray, weight: np.ndarray, alpha: np.ndarray) -> np.ndarray:
    return run(x, weight, alpha)[0]


# revision 7
# speedup vs baseline: 1.2163x; 1.0598x over previous
"""Binary 3x3 conv (sign(x) * sign(w) conv, scaled by alpha) on 8 TRN2 NeuronCores.

Strategy
--------
- Data-parallel over batch: 32 images -> 4 per core; weights replicated.
- Conv lowered to 9 shifted matmuls accumulating in PSUM, contracting over
  input channels (C=256) placed on SBUF partitions (2 chunks of 128).
- sign(x), sign(w) computed on-chip (ScalarE Sign activation) directly into
  fp8e4m3 — ±1/0 are exact in fp8, products are ±1/0, PSUM accumulates in
  fp32, sums ≤ 2304 are exact integers -> bit-exact result.
- fp8 DoubleRow perf mode packs both 128-channel chunks into one matmul
  (effective K=256, 2 MACs/cell/cycle) -> 504 matmuls/core at ~194ns issue
  rate = ~98us PE floor (the fp8 roofline for direct conv).
- Activation planes stored with a single pad column per row (57 wide): a
  row's right halo IS the next row's left pad, so every 3x3 tap window is a
  *contiguous* 1-D span of the flattened plane. One garbage output column
  per row (c=0), dropped during PSUM->SBUF eviction.
- Latency hiding: per-image pad tiles + per-tap weight tiles so the first
  matmul only waits for tap-0 weights and image 0; x loads chunked so sign
  overlaps DMA; dummy matmuls on a zero scratch tile keep the PE HAM clock
  gate warm through the prologue; PSUM evictions all on VectorE (its FIFO
  has nothing else, so banks free deterministically).
"""

import numpy as np

import concourse.bacc as bacc
import concourse.bass as bass
import concourse.mybir as mybir
from concourse import tile
from concourse.bass_utils import run_bass_kernel_spmd

N_CORES = 8
B, C, H, W = 32, 256, 56, 56
BP = B // N_CORES  # images per core
O = 256
PW = W + 1  # padded row width: one shared pad column per row
PLANE = 3312  # fp8 elems per (img, cc) plane; 58*57=3306 used, %16==0
GUARD = 16  # header so the (dy=-1,dx=-1) tap of cc0 stays in-bounds
PAD_FREE = GUARD + 2 * PLANE

ROWS_PER_TILE = 8
NT = H // ROWS_PER_TILE  # 7 pixel tiles per image
FD = ROWS_PER_TILE * PW  # 456 matmul free dim (<=512: one PSUM bank)

XCH = 2  # DMA/sign chunks per (img, cc) plane
CH_ROWS = H // XCH
N_WARMUP_MM = 48  # dummy matmuls bridging the prologue at ~194ns each

F8 = mybir.dt.float8e4
F32 = mybir.dt.float32

_compiled = None


def _build():
    nc = bacc.Bacc("TRN2", target_bir_lowering=False, debug=False, num_devices=N_CORES)

    x_dram = nc.dram_tensor("x", [BP, C, H, W], F32, kind="ExternalInput")
    wt_dram = nc.dram_tensor("wt", [C, 9, O], F32, kind="ExternalInput")
    alpha_dram = nc.dram_tensor("alpha", [1], F32, kind="ExternalInput")
    out_dram = nc.dram_tensor("out", [BP, O, H, W], F32, kind="ExternalOutput")

    with tile.TileContext(nc) as tc:
        with (
            tc.tile_pool(name="const", bufs=1) as const_pool,
            tc.tile_pool(name="xin", bufs=6) as xin_pool,
            tc.tile_pool(name="wstage", bufs=3) as wstage_pool,
            tc.tile_pool(name="oplane", bufs=3) as out_pool,
            tc.tile_pool(name="psum", bufs=7, space=bass.MemorySpace.PSUM) as psum_pool,
            tc.tile_pool(name="wpsum", bufs=1, space=bass.MemorySpace.PSUM) as wpsum_pool,
        ):
            # --- PE warm-up: matmuls on a zeroed scratch tile, no data deps
            # (pair stride must be 16-aligned: pad the scratch to 464 wide)
            warm = const_pool.tile([128, 2, 464], F8, name="warm")
            nc.gpsimd.memset(warm[:], 0)
            wps = wpsum_pool.tile([128, FD], F32, name="wps")
            for _ in range(N_WARMUP_MM):
                nc.tensor.matmul(
                    wps[:],
                    warm[:, :, 0:128],
                    warm[:, :, 0:FD],
                    start=True,
                    stop=True,
                    perf_mode=mybir.MatmulPerfMode.DoubleRow,
                )

            # alpha broadcast to all 128 partitions (scalar-engine DMA ring)
            alpha_sb = const_pool.tile([128, 1], F32, name="alpha_sb")
            nc.scalar.dma_start(alpha_sb[:], alpha_dram.ap().partition_broadcast(128))

            # per-tap weight tiles: [c_part, cc, o] f32 -> sign -> fp8.
            # wt HBM layout is [c, s, o]: c stride 9*O, cc stride 128*9*O.
            w8s = [
                const_pool.tile([128, 2, O], F8, name=f"w8_{s}") for s in range(9)
            ]

            def load_tap_weights(s):
                wstage = wstage_pool.tile([128, 2, O], F32, name="wstage", tag="ws")
                src = bass.AP(wt_dram, s * O, [[9 * O, 128], [128 * 9 * O, 2], [1, O]])
                nc.sync.dma_start(wstage[:], src)
                nc.scalar.sign(w8s[s][:], wstage[:])

            # per-image padded fp8 activation planes (both cc chunks in one
            # tile: the DoubleRow rhs AP needs a fixed stride between chunks)
            pads = [
                const_pool.tile([128, PAD_FREE], F8, name=f"pad{img}")
                for img in range(BP)
            ]
            for img in range(BP):
                ph, pstep = pads[img][:].tensor, pads[img][:].ap[0][0]
                for cc in range(2):
                    base = GUARD + cc * PLANE
                    # top pad row (+ leading guard elem); bottom pad row
                    # (+ the sliver the widest tap reads); left pad column
                    nc.gpsimd.memset(
                        bass.AP(ph, base - 1, [[pstep, 128], [1, PW + 1]]), 0
                    )
                    nc.gpsimd.memset(
                        bass.AP(ph, base + 57 * PW, [[pstep, 128], [1, PLANE - 57 * PW]]),
                        0,
                    )
                    nc.gpsimd.memset(
                        bass.AP(ph, base + PW, [[pstep, 128], [PW, H], [1, 1]]), 0
                    )

            def load_image(img):
                ph, pstep = pads[img][:].tensor, pads[img][:].ap[0][0]
                for cc in range(2):
                    for ch in range(XCH):
                        h0 = ch * CH_ROWS
                        xin = xin_pool.tile([128, CH_ROWS, W], F32, name="xin", tag="xi")
                        nc.sync.dma_start(
                            xin[:],
                            x_dram[img, cc * 128 : (cc + 1) * 128, h0 : h0 + CH_ROWS],
                        )
                        dst = bass.AP(
                            ph,
                            GUARD + cc * PLANE + (h0 + 1) * PW + 1,
                            [[pstep, 128], [PW, CH_ROWS], [1, W]],
                        )
                        nc.scalar.sign(dst, xin[:])

            # DMA priority order on the sync ring (FIFO): tap-0 weights,
            # image 0, remaining weights, images 1-3.
            load_tap_weights(0)
            load_image(0)
            for s in range(1, 9):
                load_tap_weights(s)
            for img in range(1, BP):
                load_image(img)

            # conv: 9 shifted fp8 DoubleRow matmuls per output tile
            for img in range(BP):
                ph, pstep = pads[img][:].tensor, pads[img][:].ap[0][0]
                for oc in range(2):
                    psums = [
                        psum_pool.tile([128, FD], F32, name="ps", tag="ps")
                        for _ in range(NT)
                    ]
                    for s in range(9):
                        dy, dx = s // 3 - 1, s % 3 - 1
                        wts = w8s[s][:]
                        lhsT = bass.AP(
                            wts.tensor,
                            oc * 128,
                            [[wts.ap[0][0], 128], [O, 2], [1, 128]],
                        )
                        for t in range(NT):
                            rhs = bass.AP(
                                ph,
                                GUARD + (ROWS_PER_TILE * t + 1 + dy) * PW + dx,
                                [[pstep, 128], [PLANE, 2], [1, FD]],
                            )
                            nc.tensor.matmul(
                                psums[t][:],
                                lhsT,
                                rhs,
                                start=(s == 0),
                                stop=(s == 8),
                                perf_mode=mybir.MatmulPerfMode.DoubleRow,
                            )
                    # evict on DVE only: drop the garbage column, scale by alpha
                    oplane = out_pool.tile([128, H, W], F32, name="oplane")
                    for t in range(NT):
                        pbase = psums[t][:]
                        src = bass.AP(
                            pbase.tensor,
                            pbase.offset + 1,
                            [[pbase.ap[0][0], 128], [PW, ROWS_PER_TILE], [1, W]],
                        )
                        dst = oplane[:, ROWS_PER_TILE * t : ROWS_PER_TILE * (t + 1), :]
                        nc.vector.tensor_scalar_mul(dst, src, alpha_sb[:, 0:1])
                    # split the store so it starts before the last eviction
                    half = (NT // 2) * ROWS_PER_TILE  # rows 0..23 / 24..55
                    och = out_dram[img, oc * 128 : (oc + 1) * 128]
                    nc.sync.dma_start(och[:, :half, :], oplane[:, :half, :])
                    nc.sync.dma_start(och[:, half:, :], oplane[:, half:, :])

    nc.compile()
    return nc


def _get_compiled():
    global _compiled
    if _compiled is None:
        _compiled = _build()
    return _compiled


def run(x: np.ndarray, weight: np.ndarray, alpha: np.ndarray, **kw):
    nc = _get_compiled()
    # [o,c,ky,kx] -> [c, ky*3+kx, o] so channels land on partitions directly
    wt = np.ascontiguousarray(weight.transpose(1, 2, 3, 0).reshape(C, 9, O)).astype(
        np.float32
    )
    x = np.ascontiguousarray(x, dtype=np.float32)
    alpha = np.ascontiguousarray(alpha, dtype=np.float32)
    in_maps = [
        {"x": x[i * BP : (i + 1) * BP], "wt": wt, "alpha": alpha}
        for i in range(N_CORES)
    ]
    res = run_bass_kernel_spmd(nc, in_maps, list(range(N_CORES)), **kw)
    return np.concatenate([r["out"] for r in res.results], axis=0), res


def kernel(x: np.ndarray, weight: np.ndarray, alpha: np.ndarray) -> np.ndarray:
    return run(x, weight, alpha)[0]
